# revision 3
# baseline (speedup 1.0000x reference)
"""LoRA linear layer (out = x @ (W + s*A@B) + bias) on 8 Trainium2 NeuronCores.

Sharding: data-parallel over rows of x (M = 4*2048 = 8192 -> 1024 rows/core);
each core computes its row-slice against the full weight matrix.

The LoRA update is folded into the weight on the host (W' = W + s*A@B, a
0.5 GFLOP rank-16 update) and everything is cast to bf16 there, so the
device kernel is a pure bf16 GEMM at the PE streaming floor:

  - stationary = W' tile [128k x 128n] bf16, moving = xT [128k x 512m] bf16;
    32 k-tile matmuls accumulate each [128n x 512m] fp32 PSUM tile (out is
    computed transposed; the host transposes it back). bf16 keeps FWL
    weight loads enabled so LDWEIGHTS hides under the 512-cycle matmuls.
  - W' streams in per-n-slab [128 x 32kt x 128n] (1 MiB contiguous DMAs,
    6-deep prefetch); x (8 MiB bf16) is SBUF-resident, loaded per k-tile
    so the first n-slab's matmuls start after ~1 us.
  - bias is added during the PSUM -> SBUF copy on the scalar engine
    (per-partition bias = per-output-channel in the transposed layout).

bf16 end-to-end max rel err vs the fp32 reference is ~2.4e-3 (8x inside
the 2e-2 gate).
"""
import numpy as np
import ml_dtypes

import concourse.bass as bass
import concourse.tile as tile
from concourse import bacc, mybir
from concourse.bass_utils import run_bass_kernel_spmd

P = 128
N_CORES = 8
BATCH, SEQ = 4, 2048
D_IN, D_OUT, RANK = 4096, 4096, 16
M_FULL = BATCH * SEQ          # 8192
M_C = M_FULL // N_CORES       # 1024 rows per core
KT = D_IN // P                # 32 k-tiles
MC = M_C // 512               # 2 moving chunks of 512
NT = D_OUT // P               # 32 n-tiles (one 128-col W slab each)
F32 = mybir.dt.float32
BF16 = mybir.dt.bfloat16
BF16_NP = ml_dtypes.bfloat16

_NC_CACHE = None


def _emit_body(nc, pools, aps, sb, rep):
    """Emit one full pass of the kernel (rep > 0 only used for timing)."""
    singles, w_pool, out_pool, psum_pool = pools
    x_d, w_d, bias_d, outt_d = aps
    xT = sb["xT"]

    # ---- first n-slab streamed per k-tile so the PE starts immediately ----
    w0 = w_pool.tile([P, KT, P], BF16, tag="wt", name=f"w0_{rep}")
    nc.sync.dma_start(out=xT[:, 0, :], in_=x_d[:, 0, :])
    nc.sync.dma_start(out=w0[:, 0, :], in_=w_d[0, :, 0, :])
    if "bias_cols" not in sb:
        sb["bias_cols"] = singles.tile([P, NT], F32, name="bias_cols")
    bias_cols = sb["bias_cols"]
    nc.sync.dma_start(out=bias_cols, in_=bias_d)
    ps0 = [psum_pool.tile([P, 512], F32, tag="ps", name=f"ps_{rep}_0_{mc}")
           for mc in range(MC)]
    for kt in range(KT):
        if kt > 0:
            nc.sync.dma_start(out=xT[:, kt, :], in_=x_d[:, kt, :])
            nc.sync.dma_start(out=w0[:, kt, :], in_=w_d[0, :, kt, :])
        for mc in range(MC):
            nc.tensor.matmul(
                ps0[mc],
                w0[:, kt, :],
                xT[:, kt, mc * 512:(mc + 1) * 512],
                start=(kt == 0),
                stop=(kt == KT - 1),
            )
    for mc in range(MC):
        ob = out_pool.tile([P, 512], F32, tag="ob", name=f"ob_{rep}_0_{mc}")
        nc.scalar.activation(
            ob, ps0[mc],
            mybir.ActivationFunctionType.Identity,
            bias=bias_cols[:, 0:1],
        )
        nc.sync.dma_start(
            out=outt_d[0:P, mc * 512:(mc + 1) * 512], in_=ob)

    # ---- remaining n-slabs: whole-slab DMA, k-inner accumulation ----
    for nt in range(1, NT):
        wt = w_pool.tile([P, KT, P], BF16, tag="wt", name=f"wt_{rep}_{nt}")
        nc.sync.dma_start(out=wt, in_=w_d[nt])
        psums = [psum_pool.tile([P, 512], F32, tag="ps",
                                name=f"ps_{rep}_{nt}_{mc}")
                 for mc in range(MC)]
        for kt in range(KT):
            for mc in range(MC):
                nc.tensor.matmul(
                    psums[mc],
                    wt[:, kt, :],
                    xT[:, kt, mc * 512:(mc + 1) * 512],
                    start=(kt == 0),
                    stop=(kt == KT - 1),
                )
        for mc in range(MC):
            ob = out_pool.tile([P, 512], F32, tag="ob",
                               name=f"ob_{rep}_{nt}_{mc}")
            nc.scalar.activation(
                ob, psums[mc],
                mybir.ActivationFunctionType.Identity,
                bias=bias_cols[:, nt:nt + 1],
            )
            nc.sync.dma_start(
                out=outt_d[nt * P:(nt + 1) * P, mc * 512:(mc + 1) * 512],
                in_=ob,
            )


def _build_nc(n_reps=1):
    nc = bacc.Bacc("TRN2", target_bir_lowering=False, debug=False,
                   num_devices=N_CORES)
    # x slice pre-transposed+tiled on host: [128 p, 32 kt, 1024 m] bf16
    x_d = nc.dram_tensor("xt", [P, KT, M_C], BF16, kind="ExternalInput").ap()
    # W' pre-tiled on host: [32 nt, 128 p, 32 kt, 128 n] bf16 (slab-contig)
    w_d = nc.dram_tensor("w", [NT, P, KT, P], BF16, kind="ExternalInput").ap()
    # bias striped on host: bias_cols[p, nt] = bias[nt*128 + p]
    bias_d = nc.dram_tensor("bias", [P, NT], F32, kind="ExternalInput").ap()
    outt_d = nc.dram_tensor("outt", [D_OUT, M_C], F32,
                            kind="ExternalOutput").ap()

    with tile.TileContext(nc) as tc:
        with (
            tc.tile_pool(name="singles", bufs=1) as singles,
            tc.tile_pool(name="wts", bufs=6) as w_pool,
            tc.tile_pool(name="outs", bufs=6) as out_pool,
            tc.tile_pool(name="psum", bufs=8, space="PSUM") as psum_pool,
        ):
            sb = {
                "xT": singles.tile([P, KT, M_C], BF16, name="xT"),
            }
            pools = (singles, w_pool, out_pool, psum_pool)
            aps = (x_d, w_d, bias_d, outt_d)
            for rep in range(n_reps):
                _emit_body(nc, pools, aps, sb, rep)

    nc.compile()
    return nc


def get_nc():
    global _NC_CACHE
    if _NC_CACHE is None:
        _NC_CACHE = _build_nc()
    return _NC_CACHE


def make_in_maps(x, W, bias, lora_A, lora_B, scaling):
    x2 = np.asarray(x, dtype=np.float32).reshape(M_FULL, D_IN)
    s = np.float32(np.asarray(scaling).astype(np.float64))
    a = np.asarray(lora_A, dtype=np.float32)
    b = np.asarray(lora_B, dtype=np.float32)
    wp = (np.asarray(W, dtype=np.float32) + s * (a @ b)).astype(BF16_NP)
    # w_tiled[nt, p, kt, n] = W'[kt*128 + p, nt*128 + n]
    w_tiled = np.ascontiguousarray(
        wp.reshape(KT, P, NT, P).transpose(2, 1, 0, 3))
    bias_cols = np.ascontiguousarray(
        np.asarray(bias, dtype=np.float32).reshape(NT, P).T)
    in_maps = []
    for c in range(N_CORES):
        xt = x2[c * M_C:(c + 1) * M_C].T.astype(BF16_NP)     # [4096, 1024]
        xt_tiled = np.ascontiguousarray(
            xt.reshape(KT, P, M_C).transpose(1, 0, 2))       # [128, 32, 1024]
        in_maps.append({
            "xt": xt_tiled,
            "w": w_tiled,
            "bias": bias_cols,
        })
    return in_maps


def assemble_output(results):
    """results: list of per-core dicts with 'outt' [D_OUT, M_C]."""
    out = np.concatenate(
        [results[c]["outt"].T for c in range(N_CORES)], axis=0)
    return np.ascontiguousarray(out).reshape(BATCH, SEQ, D_OUT)


def kernel(x, W, bias, lora_A, lora_B, scaling):
    nc = get_nc()
    in_maps = make_in_maps(x, W, bias, lora_A, lora_B, scaling)
    res = run_bass_kernel_spmd(nc, in_maps, core_ids=list(range(N_CORES)))
    return assemble_output(res.results)
